# revision 11
# baseline (speedup 1.0000x reference)
"""LightweightConv1dTBC forward as a Trainium2 Bass kernel.

Math: y[t, b, c] = sum_k softmax(weight)[head(c), k] * x[t + k - PAD, b, c] + bias[c]
with T=2048, B=32, C=1024, H=16 heads (R = C//H = 64 channels each), K=31, PAD=15.

Strategy:
- Data-parallel over batch: 8 cores x 4 sequences each.
- The depthwise time-conv is cast as a banded-Toeplitz matmul on the
  TensorEngine: for each head h, a constant stationary matrix
  A_h[p, m] = w[h, p - m] (0 <= p-m < K), shape (128, 98), computed on host
  from the (tiny) softmaxed kernel. An input tile X of 128 consecutive
  timesteps (partitions) x (head, batch, 64ch) (free dim, head-major so each
  head's moving operand is 256 contiguous floats) yields 98 output timesteps
  per matmul: Y[m, (b,c)] = sum_p A_h[p, m] X[p, (b,c)].
- T is chunked 128-in -> 98-out; consecutive chunks share a 30-row halo that
  is copied SBUF->SBUF on the SWDGE ring instead of re-read from HBM.
- Matmuls run in float32r (TF32-class, 1 cycle/row sustained when warm);
  two heads share each PSUM bank (contiguous 256-float halves) so all 16
  head-matmuls of a chunk can be in flight across the 8 banks, keeping the
  PE queue deep. Bank drains (98x512) alternate between the vector and
  scalar engines.
- Input DMAs ride the sync HWDGE ring, output DMAs the scalar ring; x and y
  are (de)interleaved to/from head-major layout on the host.
"""

import numpy as np

from concourse import bacc, tile
from concourse.bass_utils import run_bass_kernel_spmd
import concourse.mybir as mybir

T, B, C, H, K, PAD = 2048, 32, 1024, 16, 31, 15
R = C // H                      # channels per head
NCORES = 8
BL = B // NCORES                # batch per core
CH_IN = 128                     # input rows per chunk (partition dim)
CH_OUT = CH_IN - (K - 1)        # output rows per chunk = 98
NCH = (T + CH_OUT - 1) // CH_OUT  # 21 chunks
HALO = 2 * PAD                  # 30 rows shared between consecutive chunks
F32 = mybir.dt.float32
F32R = mybir.dt.float32r


def _build_nc(with_bias: bool):
    nc = bacc.Bacc("TRN2", target_bir_lowering=False, debug=False)
    x_d = nc.dram_tensor("x", [T, H, BL, R], F32R, kind="ExternalInput")
    a_d = nc.dram_tensor("a", [CH_IN, H * CH_OUT], F32R, kind="ExternalInput")
    if with_bias:
        b_d = nc.dram_tensor("bias", [CH_IN, H, BL, R], F32, kind="ExternalInput")
    y_d = nc.dram_tensor("y", [T, H, BL, R], F32, kind="ExternalOutput")

    with tile.TileContext(nc) as tc:
        with (
            tc.tile_pool(name="const", bufs=1) as cpool,
            tc.tile_pool(name="xin", bufs=5) as xpool,
            tc.tile_pool(name="yout", bufs=3) as ypool,
            tc.tile_pool(name="ps", bufs=8, space="PSUM") as pspool,
        ):
            A = cpool.tile([CH_IN, H * CH_OUT], F32R)
            nc.sync.dma_start(A[:], a_d[:])
            if with_bias:
                BIAS = cpool.tile([CH_IN, H, BL, R], F32)
                nc.sync.dma_start(BIAS[:], b_d[:])

            prevX = None
            for i in range(NCH):
                t0 = i * CH_OUT
                out_m = min(CH_OUT, T - t0)
                s = t0 - PAD
                # fresh rows: everything this chunk needs that the previous
                # chunk's tile doesn't already hold (its last HALO rows)
                flo = s if i == 0 else s + HALO
                lo, hi = max(0, flo), min(T, s + CH_IN)
                plo, phi = lo - s, hi - s

                X = xpool.tile([CH_IN, H, BL, R], F32R, tag="X")
                if i == 0:
                    nc.vector.memset(X[0:plo].bitcast(F32), 0.0)
                else:
                    # halo: last 30 rows of the previous tile, via SBUF->SBUF
                    # DMA on the SWDGE ring (saves HBM read bandwidth)
                    nc.gpsimd.dma_start(X[0:HALO], prevX[CH_OUT:CH_IN])
                if phi < CH_IN:
                    # engine ops need a 32-aligned base partition; memset the
                    # whole aligned tail (the DMA below rewrites the overlap)
                    nc.vector.memset(X[(phi // 32) * 32:CH_IN].bitcast(F32), 0.0)
                nc.sync.dma_start(X[plo:phi], x_d[lo:hi])
                prevX = X

                Y = ypool.tile([CH_OUT, H, BL, R], F32, tag="Y")
                for hp in range(H // 2):        # head pairs share a PSUM bank
                    ps = pspool.tile([CH_OUT, 2, BL, R], F32, tag="ps")
                    for j in range(2):
                        h = 2 * hp + j
                        nc.tensor.matmul(
                            ps[:, j],
                            A[:, h * CH_OUT:(h + 1) * CH_OUT],
                            X[:, h],
                            start=True,
                            stop=True,
                        )
                    if with_bias:
                        nc.vector.tensor_tensor(
                            out=Y[0:out_m, 2 * hp:2 * hp + 2],
                            in0=ps[0:out_m],
                            in1=BIAS[0:out_m, 2 * hp:2 * hp + 2],
                            op=mybir.AluOpType.add,
                        )
                    else:
                        if hp % 2 == 0:
                            nc.vector.tensor_copy(
                                out=Y[0:out_m, 2 * hp:2 * hp + 2],
                                in_=ps[0:out_m],
                            )
                        else:
                            nc.scalar.copy(
                                out=Y[0:out_m, 2 * hp:2 * hp + 2],
                                in_=ps[0:out_m],
                            )
                nc.scalar.dma_start(y_d[t0:t0 + out_m], Y[0:out_m])

    nc.compile()
    return nc


def _toeplitz(weight: np.ndarray) -> np.ndarray:
    """Softmax the (H,1,K) kernel and build the (128, H*98) stationary matrix."""
    wl = weight[:, 0, :].astype(np.float32)
    e = np.exp(wl - wl.max(axis=-1, keepdims=True))
    w = (e / e.sum(axis=-1, keepdims=True)).astype(np.float32)  # (H, K)
    a = np.zeros((H, CH_IN, CH_OUT), dtype=np.float32)
    m = np.arange(CH_OUT)[None, :]
    p = np.arange(CH_IN)[:, None]
    k = p - m                                                   # (128, 98)
    mask = (k >= 0) & (k < K)
    for h in range(H):
        a[h][mask] = w[h][k[mask]]
    # (CH_IN, H, CH_OUT) -> head h occupies columns [h*98, (h+1)*98)
    return np.ascontiguousarray(a.transpose(1, 0, 2).reshape(CH_IN, H * CH_OUT))


def kernel(x: np.ndarray, weight: np.ndarray, bias: np.ndarray, **run_kwargs):
    x = np.ascontiguousarray(x, dtype=np.float32)
    a_all = _toeplitz(np.asarray(weight))
    bias = np.asarray(bias, dtype=np.float32)
    with_bias = bool(np.any(bias))

    nc = _build_nc(with_bias)

    in_maps = []
    for i in range(NCORES):
        xs = x[:, i * BL:(i + 1) * BL, :].reshape(T, BL, H, R)
        m = {"x": np.ascontiguousarray(xs.transpose(0, 2, 1, 3)), "a": a_all}
        if with_bias:
            bb = np.broadcast_to(bias.reshape(H, R), (CH_IN, BL, H, R))
            m["bias"] = np.ascontiguousarray(bb.transpose(0, 2, 1, 3))
        in_maps.append(m)

    res = run_bass_kernel_spmd(nc, in_maps, core_ids=list(range(NCORES)), **run_kwargs)

    y = np.empty((T, B, C), dtype=np.float32)
    for i in range(NCORES):
        # y comes back head-major (T, H, BL, R) -> (T, BL, C)
        yi = res.results[i]["y"].transpose(0, 2, 1, 3).reshape(T, BL, C)
        y[:, i * BL:(i + 1) * BL, :] = yi
    if run_kwargs:
        return y, res
    return y


# revision 12
# speedup vs baseline: 1.5084x; 1.5084x over previous
"""LightweightConv1dTBC forward as a Trainium2 Bass kernel.

Math: y[t, b, c] = sum_k softmax(weight)[head(c), k] * x[t + k - PAD, b, c] + bias[c]
with T=2048, B=32, C=1024, H=16 heads (R = C//H = 64 channels each), K=31, PAD=15.

Strategy:
- Data-parallel over batch: 8 cores x 4 sequences each.
- The depthwise time-conv is cast as a banded-Toeplitz matmul on the
  TensorEngine: for each head h, a constant stationary matrix
  A_h[p, m] = w[h, p - m] (0 <= p-m < K), shape (128, 98), computed on host
  from the (tiny) softmaxed kernel. An input tile X of 128 consecutive
  timesteps (partitions) x (head, batch, 64ch) (free dim, head-major so each
  head's moving operand is 256 contiguous floats) yields 98 output timesteps
  per matmul: Y[m, (b,c)] = sum_p A_h[p, m] X[p, (b,c)].
- T is chunked 128-in -> 98-out; consecutive chunks share a 30-row halo that
  is copied SBUF->SBUF on the SWDGE ring instead of re-read from HBM.
- Matmuls run in float32r (TF32-class, 1 cycle/row sustained when warm);
  two heads share each PSUM bank (contiguous 256-float halves) so all 16
  head-matmuls of a chunk can be in flight across the 8 banks, keeping the
  PE queue deep. Bank drains (98x512) alternate between the vector and
  scalar engines.
- Input DMAs ride the sync HWDGE ring, output DMAs the scalar ring; x and y
  are (de)interleaved to/from head-major layout on the host.
"""

import numpy as np

from concourse import bacc, tile
from concourse.bass_utils import run_bass_kernel_spmd
import concourse.mybir as mybir

T, B, C, H, K, PAD = 2048, 32, 1024, 16, 31, 15
R = C // H                      # channels per head
NCORES = 8
BL = B // NCORES                # batch per core
CH_IN = 128                     # input rows per chunk (partition dim)
CH_OUT = CH_IN - (K - 1)        # output rows per chunk = 98
NCH = (T + CH_OUT - 1) // CH_OUT  # 21 chunks
HALO = 2 * PAD                  # 30 rows shared between consecutive chunks
F32 = mybir.dt.float32
F32R = mybir.dt.float32r
F16 = mybir.dt.float16


def _build_nc(with_bias: bool):
    nc = bacc.Bacc("TRN2", target_bir_lowering=False, debug=False)
    x_d = nc.dram_tensor("x", [T, H, BL, R], F16, kind="ExternalInput")
    a_d = nc.dram_tensor("a", [CH_IN, H * CH_OUT], F16, kind="ExternalInput")
    if with_bias:
        b_d = nc.dram_tensor("bias", [CH_IN, H, BL, R], F32, kind="ExternalInput")
    y_d = nc.dram_tensor("y", [T, H, BL, R], F32, kind="ExternalOutput")

    with tile.TileContext(nc) as tc:
        with (
            tc.tile_pool(name="const", bufs=1) as cpool,
            tc.tile_pool(name="xin", bufs=5) as xpool,
            tc.tile_pool(name="yout", bufs=3) as ypool,
            tc.tile_pool(name="ps", bufs=8, space="PSUM") as pspool,
        ):
            A = cpool.tile([CH_IN, H * CH_OUT], F16)
            nc.sync.dma_start(A[:], a_d[:])
            if with_bias:
                BIAS = cpool.tile([CH_IN, H, BL, R], F32)
                nc.sync.dma_start(BIAS[:], b_d[:])

            prevX = None
            for i in range(NCH):
                t0 = i * CH_OUT
                out_m = min(CH_OUT, T - t0)
                s = t0 - PAD
                # fresh rows: everything this chunk needs that the previous
                # chunk's tile doesn't already hold (its last HALO rows)
                flo = s if i == 0 else s + HALO
                lo, hi = max(0, flo), min(T, s + CH_IN)
                plo, phi = lo - s, hi - s

                X = xpool.tile([CH_IN, H, BL, R], F16, tag="X")
                if i == 0:
                    nc.vector.memset(X[0:plo], 0.0)
                else:
                    # halo: last 30 rows of the previous tile, via SBUF->SBUF
                    # DMA on the SWDGE ring (saves HBM read bandwidth)
                    nc.gpsimd.dma_start(X[0:HALO], prevX[CH_OUT:CH_IN])
                if phi < CH_IN:
                    # engine ops need a 32-aligned base partition; memset the
                    # whole aligned tail (the DMA below rewrites the overlap)
                    nc.vector.memset(X[(phi // 32) * 32:CH_IN], 0.0)
                nc.sync.dma_start(X[plo:phi], x_d[lo:hi])
                prevX = X

                Y = ypool.tile([CH_OUT, H, BL, R], F32, tag="Y")
                for hp in range(H // 2):        # head pairs share a PSUM bank
                    ps = pspool.tile([CH_OUT, 2, BL, R], F32, tag="ps")
                    for j in range(2):
                        h = 2 * hp + j
                        nc.tensor.matmul(
                            ps[:, j],
                            A[:, h * CH_OUT:(h + 1) * CH_OUT],
                            X[:, h],
                            start=True,
                            stop=True,
                        )
                    if with_bias:
                        nc.vector.tensor_tensor(
                            out=Y[0:out_m, 2 * hp:2 * hp + 2],
                            in0=ps[0:out_m],
                            in1=BIAS[0:out_m, 2 * hp:2 * hp + 2],
                            op=mybir.AluOpType.add,
                        )
                    else:
                        if hp % 2 == 0:
                            nc.vector.tensor_copy(
                                out=Y[0:out_m, 2 * hp:2 * hp + 2],
                                in_=ps[0:out_m],
                            )
                        else:
                            nc.scalar.copy(
                                out=Y[0:out_m, 2 * hp:2 * hp + 2],
                                in_=ps[0:out_m],
                            )
                nc.scalar.dma_start(y_d[t0:t0 + out_m], Y[0:out_m])

    nc.compile()
    return nc


def _toeplitz(weight: np.ndarray) -> np.ndarray:
    """Softmax the (H,1,K) kernel and build the (128, H*98) stationary matrix."""
    wl = weight[:, 0, :].astype(np.float32)
    e = np.exp(wl - wl.max(axis=-1, keepdims=True))
    w = (e / e.sum(axis=-1, keepdims=True)).astype(np.float32)  # (H, K)
    a = np.zeros((H, CH_IN, CH_OUT), dtype=np.float32)
    m = np.arange(CH_OUT)[None, :]
    p = np.arange(CH_IN)[:, None]
    k = p - m                                                   # (128, 98)
    mask = (k >= 0) & (k < K)
    for h in range(H):
        a[h][mask] = w[h][k[mask]]
    # (CH_IN, H, CH_OUT) -> head h occupies columns [h*98, (h+1)*98)
    return np.ascontiguousarray(a.transpose(1, 0, 2).reshape(CH_IN, H * CH_OUT))


def kernel(x: np.ndarray, weight: np.ndarray, bias: np.ndarray, **run_kwargs):
    x = np.ascontiguousarray(x, dtype=np.float32)
    a_all = _toeplitz(np.asarray(weight))
    bias = np.asarray(bias, dtype=np.float32)
    with_bias = bool(np.any(bias))

    nc = _build_nc(with_bias)

    in_maps = []
    for i in range(NCORES):
        xs = x[:, i * BL:(i + 1) * BL, :].reshape(T, BL, H, R)
        m = {"x": np.ascontiguousarray(xs.transpose(0, 2, 1, 3)).astype(np.float16),
             "a": a_all.astype(np.float16)}
        if with_bias:
            bb = np.broadcast_to(bias.reshape(H, R), (CH_IN, BL, H, R))
            m["bias"] = np.ascontiguousarray(bb.transpose(0, 2, 1, 3))
        in_maps.append(m)

    res = run_bass_kernel_spmd(nc, in_maps, core_ids=list(range(NCORES)), **run_kwargs)

    y = np.empty((T, B, C), dtype=np.float32)
    for i in range(NCORES):
        # y comes back head-major (T, H, BL, R) -> (T, BL, C)
        yi = res.results[i]["y"].transpose(0, 2, 1, 3).reshape(T, BL, C)
        y[:, i * BL:(i + 1) * BL, :] = yi
    if run_kwargs:
        return y, res
    return y
